# revision 1
# baseline (speedup 1.0000x reference)
"""ConformerEncoder kernel for 8 NeuronCores.

Distribution (per sharding hint): data-parallel over batch across 4 groups
(mesh axis 'b'); within each group, 2-way tensor-parallel (mesh axis 'm'):
attention heads split 8->4, FFN hidden columns split 2048->1024 with psum
after the second linear, conv module split on channels with psum after the
pointwise projection. BatchNorm batch statistics are reduced over 'b'.

Self-contained: hardcodes shapes from the problem spec.
"""
import numpy as np
import jax
import jax.numpy as jnp
from jax import lax
from jax.sharding import Mesh, PartitionSpec as P
from jax.experimental.shard_map import shard_map
from functools import partial

NUM_LAYERS = 2
D_MODEL = 512
D_FFN = 2048
NHEAD = 8
KERNEL = 31
HEAD_DIM = D_MODEL // NHEAD
EPS = 1e-5
B, S = 4, 1024

MB = 4   # batch shards
MT = 2   # tensor-parallel shards
H_LOC = NHEAD // MT        # 4 heads per TP shard
F_LOC = D_FFN // MT        # 1024 ffn cols per TP shard
C_LOC = D_MODEL // MT      # 256 conv channels per TP shard


def _layer_norm(x, g, b):
    mu = x.mean(-1, keepdims=True)
    var = x.var(-1, keepdims=True)
    return (x - mu) * lax.rsqrt(var + EPS) * g + b


def _rel_shift(x):
    # x: (B, H, Q, P) with P = 2*S - 1
    b, h, q, p = x.shape
    x = jnp.concatenate([jnp.zeros((b, h, q, 1), x.dtype), x], axis=-1)
    x = x.reshape(b, h, p + 1, q)[:, :, 1:, :].reshape(b, h, q, p)
    return x[..., : p // 2 + 1]


def _ffn_tp(x, w1, b1, w2, b2, mi):
    # w1: (F, D) rows split; w2: (D, F) cols split; psum over 'm'
    w1l = lax.dynamic_slice_in_dim(w1, mi * F_LOC, F_LOC, 0)
    b1l = lax.dynamic_slice_in_dim(b1, mi * F_LOC, F_LOC, 0)
    w2l = lax.dynamic_slice_in_dim(w2, mi * F_LOC, F_LOC, 1)
    h = jax.nn.silu(x @ w1l.T + b1l)
    part = h @ w2l.T
    return lax.psum(part, 'm') + b2


def _attn_tp(x, pos, p, mi):
    b, s, e = x.shape
    hd = HEAD_DIM
    # QKV for my heads: attn_in_w rows grouped as (H, 3, hd, D)
    w_in = p['attn_in_w'].reshape(NHEAD, 3 * hd, D_MODEL)
    w_in_l = lax.dynamic_slice_in_dim(w_in, mi * H_LOC, H_LOC, 0)  # (H_LOC,3hd,D)
    proj = jnp.einsum('bse,hde->bshd', x, w_in_l)          # (B,S,H_LOC,3hd)
    proj = proj.reshape(b, s, H_LOC, 3, hd)
    q, k, v = proj[..., 0, :], proj[..., 1, :], proj[..., 2, :]

    w_pos = p['pos_w'].reshape(NHEAD, hd, D_MODEL)
    w_pos_l = lax.dynamic_slice_in_dim(w_pos, mi * H_LOC, H_LOC, 0)
    pk = jnp.einsum('pe,hde->phd', pos, w_pos_l)           # (P,H_LOC,hd)

    pos_u_l = lax.dynamic_slice_in_dim(p['pos_u'], mi * H_LOC, H_LOC, 0)
    pos_v_l = lax.dynamic_slice_in_dim(p['pos_v'], mi * H_LOC, H_LOC, 0)

    q_u = (q + pos_u_l[None, None]).transpose(0, 2, 1, 3)  # (B,H_LOC,S,hd)
    q_v = (q + pos_v_l[None, None]).transpose(0, 2, 1, 3)
    matrix_ac = jnp.einsum('bhqd,bkhd->bhqk', q_u, k)
    matrix_bd = jnp.einsum('bhqd,phd->bhqp', q_v, pk)
    matrix_bd = _rel_shift(matrix_bd)
    scale = 1.0 / np.sqrt(D_MODEL).astype(np.float32)
    attn = jax.nn.softmax((matrix_ac + matrix_bd) * scale, axis=-1)
    out = jnp.einsum('bhqk,bkhd->bqhd', attn, v).reshape(b, s, H_LOC * hd)

    # out_proj columns for my heads
    w_out = p['attn_out_w'].reshape(D_MODEL, NHEAD, hd)
    w_out_l = lax.dynamic_slice_in_dim(w_out, mi * H_LOC, H_LOC, 1)
    w_out_l = w_out_l.reshape(D_MODEL, H_LOC * hd)
    part = out @ w_out_l.T
    return lax.psum(part, 'm') + p['attn_out_b']


def _conv_tp(x, p, mi):
    C = D_MODEL
    pad = (KERNEL - 1) // 2
    y = _layer_norm(x, p['conv_ln_g'], p['conv_ln_b'])
    y = y.transpose(0, 2, 1)  # (B,C,T)
    # bottleneck rows for my channels: a-rows [mi*C_LOC, +C_LOC), g-rows C+[...]
    a_w = lax.dynamic_slice_in_dim(p['bott_w'], mi * C_LOC, C_LOC, 0)
    g_w = lax.dynamic_slice_in_dim(p['bott_w'], C + mi * C_LOC, C_LOC, 0)
    a_b = lax.dynamic_slice_in_dim(p['bott_b'], mi * C_LOC, C_LOC, 0)
    g_b = lax.dynamic_slice_in_dim(p['bott_b'], C + mi * C_LOC, C_LOC, 0)
    a = jnp.einsum('oc,bct->bot', a_w, y) + a_b[None, :, None]
    g = jnp.einsum('oc,bct->bot', g_w, y) + g_b[None, :, None]
    u = a * jax.nn.sigmoid(g)  # (B, C_LOC, T)

    dw_l = lax.dynamic_slice_in_dim(p['dw_w'], mi * C_LOC, C_LOC, 0)  # (C_LOC,K)
    db_l = lax.dynamic_slice_in_dim(p['dw_b'], mi * C_LOC, C_LOC, 0)
    up = jnp.pad(u, ((0, 0), (0, 0), (pad, pad)))
    z = jnp.zeros_like(u)
    for j in range(KERNEL):
        z = z + dw_l[None, :, j:j + 1] * up[:, :, j:j + S]
    z = z + db_l[None, :, None]

    # BatchNorm: stats over (B, T) -> psum over 'b' (batch shards)
    n_loc = z.shape[0] * z.shape[2]
    n_tot = n_loc * MB
    ssum = lax.psum(z.sum(axis=(0, 2)), 'b')
    ssq = lax.psum((z * z).sum(axis=(0, 2)), 'b')
    mu = ssum / n_tot
    var = ssq / n_tot - mu * mu
    bn_g = lax.dynamic_slice_in_dim(p['bn_g'], mi * C_LOC, C_LOC, 0)
    bn_b = lax.dynamic_slice_in_dim(p['bn_b'], mi * C_LOC, C_LOC, 0)
    z = (z - mu[None, :, None]) * lax.rsqrt(var + EPS)[None, :, None]
    z = z * bn_g[None, :, None] + bn_b[None, :, None]
    z = jax.nn.silu(z)

    pw_l = lax.dynamic_slice_in_dim(p['pw_w'], mi * C_LOC, C_LOC, 1)  # (C, C_LOC)
    part = jnp.einsum('oc,bct->bot', pw_l, z)
    out = lax.psum(part, 'm') + p['pw_b'][None, :, None]
    return out.transpose(0, 2, 1)


def _layer(x, pos, p, mi):
    x = x + 0.5 * _ffn_tp(_layer_norm(x, p['ffn1_ln_g'], p['ffn1_ln_b']),
                          p['ffn1_w1'], p['ffn1_b1'], p['ffn1_w2'], p['ffn1_b2'], mi)
    skip = x
    h = _layer_norm(x, p['norm1_g'], p['norm1_b'])
    x = _attn_tp(h, pos, p, mi) + skip
    x = x + _conv_tp(x, p, mi)
    x = x + 0.5 * _ffn_tp(_layer_norm(x, p['ffn2_ln_g'], p['ffn2_ln_b']),
                          p['ffn2_w1'], p['ffn2_b1'], p['ffn2_w2'], p['ffn2_b2'], mi)
    return _layer_norm(x, p['norm2_g'], p['norm2_b'])


def _forward(src, pos_embs, params):
    # runs inside shard_map: src is the local batch shard (1, S, D)
    mi = lax.axis_index('m')
    x = src
    pos = pos_embs[0]
    for p in params['layers']:
        x = _layer(x, pos, p, mi)
    return _layer_norm(x, params['final_g'], params['final_b'])


_RUNNER = None


def _get_runner():
    global _RUNNER
    if _RUNNER is None:
        devs = np.asarray(jax.devices()[:8]).reshape(MB, MT)
        mesh = Mesh(devs, ('b', 'm'))
        fn = shard_map(
            _forward, mesh=mesh,
            in_specs=(P('b', None, None), P(None, None, None), P()),
            out_specs=P('b', None, None),
            check_rep=False,
        )
        _RUNNER = jax.jit(fn)
    return _RUNNER


def kernel(src, pos_embs, params):
    src = jnp.asarray(src, jnp.float32)
    pos_embs = jnp.asarray(pos_embs, jnp.float32)
    params = jax.tree.map(lambda a: jnp.asarray(a, jnp.float32), params)
    out = _get_runner()(src, pos_embs, params)
    return np.asarray(jax.block_until_ready(out), np.float32)
